# revision 24
# baseline (speedup 1.0000x reference)
"""BinaryLinear Trainium2 kernel: Y = X @ binarize(W).T + bias.

Shapes (hardcoded per the problem spec):
  X: [8192, 4096] f32, W: [4096, 4096] f32, bias: [4096] f32 -> Y: [8192, 4096] f32

Strategy: data-parallel over tokens across 8 NeuronCores (1024 tokens/core),
weight replicated. Host prepares transposed/quantized layouts; all O(N*K*O)
math runs on-device.

fp8 path: binarized weights are EXACT in fp8 (values in {0,1} or {+-0.5}), so
the only quantization error is X -> e4m3. DoubleRow perf mode contracts two
128-deep k-slabs per matmul instruction at the 2x fp8 rate (~152 TF/s/core
measured, 97% of peak).
  fp8    : single pass. W ships as {+-0.5}; the identity
           X @ Wbin.T = X @ (Wbin - 0.5).T + 0.5*rowsum(X)
           moves the common-mode half of the quantization error into an
           exact per-token f32 correction c = 0.5*rowsum(X) (host-computed
           O(N*K) prep), added during the PSUM drain. rel err 1.92e-2.
  fp8x2  : X split hi+lo e4m3, two accumulating passes, W as {0,1}; rel
           err ~7e-4 at 2x matmul cost.
  f32r   : previous full-precision fallback (fp32r matmuls, bf16 W stream).

Schedule: X^T resident in SBUF fp8 (host-quantized, host-swizzled so every
DMA line is 2-4 KiB contiguous); W streamed per 512-wide out-block into a
double-buffered resident panel, prefetched one block ahead on the Sync DMA
queue; Y/bias/c ride the Activation DMA queue so write-backs never block the
W stream. Out-block 0 runs pair-outer/m-inner while X residency streams in;
later blocks run two-bank-interleaved m-outer so each PSUM bank drains (one
DVE scalar_tensor_tensor: psum + c[token] + bias[o]) while the PE streams
the next banks. 8 warmup matmuls ramp the PE p-state during the prologue.

Compute mode via env TRNKERNEL_MODE (default fp8).
"""
import os
import sys

import numpy as np

sys.path.insert(0, "/opt/trn_rl_repo")

import concourse.bacc as bacc
import concourse.mybir as mybir
import concourse.tile as tile
from concourse.bass_utils import run_bass_kernel_spmd

N_TOKENS = 8192
IN_F = 4096
OUT_F = 4096
N_CORES = 8
TOK_C = N_TOKENS // N_CORES  # 1024 tokens per core

P = 128
K_TILES = IN_F // P          # 32
KG = 4                       # k-tiles per W/X DMA chunk
K_GROUPS = K_TILES // KG     # 8
M_TILES = TOK_C // P         # 8
OB = 512                     # out-features per block (one PSUM bank)
O_BLOCKS = OUT_F // OB       # 8

_MODE = os.environ.get("TRNKERNEL_MODE", "fp8")
_TRACE = os.environ.get("TRNKERNEL_TRACE", "0") == "1"
# timing-only probes (results intentionally wrong): "nopsum" = drains read
# SBUF instead of PSUM; "samelhs" = every matmul loads the same stationary
_PROBE = os.environ.get("TRNKERNEL_PROBE", "")
_DRAIN = os.environ.get("TRNKERNEL_DRAIN", "dve")  # "dve" | "act"

_CACHED = {}


def _install_ntff_shim():
    """Register the NTFF profile hook so trace=True yields exec_time_ns."""
    import types

    try:
        import antenv  # noqa: F401
        from trn_agent_boot.trn_boot import _ntff_profile_via_ctypes
        import concourse.bass_utils as bu

        hook = _ntff_profile_via_ctypes("/opt/axon/libaxon_pjrt.so")
        mod = types.ModuleType("antenv.axon_hooks")
        mod.get_axon_ntff_profile_hook = lambda: hook
        mod.set_axon_ntff_profile_hook = lambda h: None
        sys.modules["antenv.axon_hooks"] = mod
        bu.upload_artifacts = lambda tmpdir: tmpdir  # no artifact store here
    except Exception:
        pass


def build_fp8(mode: str):
    assert mode in ("fp8", "fp8x2")
    f8 = mybir.dt.float8e4
    two_x = mode == "fp8x2"
    N_PAIRS = K_TILES // 2       # 16 DoubleRow k-pairs

    nc = bacc.Bacc(None)
    # Host-swizzled layouts: every DMA chunk is contiguous per partition line.
    # xq[kg, p, j, t]  = X^T[(kg*KG+j)*P + p, t]            (4 KiB/line chunks)
    # wq[ob, kg, p, j, o] = Wq^T[(kg*KG+j)*P + p, ob*OB+o]  (2 KiB/line chunks)
    xq = nc.declare_dram_parameter("xq", [K_GROUPS, P, KG, TOK_C], f8, isOutput=False)
    if two_x:
        xl = nc.declare_dram_parameter("xl", [K_GROUPS, P, KG, TOK_C], f8, isOutput=False)
    wq = nc.declare_dram_parameter("wq", [O_BLOCKS, K_GROUPS, P, KG, OB], f8, isOutput=False)
    cc = nc.declare_dram_parameter("cc", [P, M_TILES], mybir.dt.float32, isOutput=False)
    bias = nc.declare_dram_parameter("bias", [OUT_F], mybir.dt.float32, isOutput=False)
    y = nc.declare_dram_parameter("y", [TOK_C, OUT_F], mybir.dt.float32, isOutput=True)

    y_v = y.rearrange("(mt p) o -> p mt o", p=P)        # [128, 8, 4096]

    with tile.TileContext(nc) as tc:
        with (
            tc.tile_pool(name="xres", bufs=1) as xres_pool,
            tc.tile_pool(name="wres", bufs=2) as wres_pool,
            tc.tile_pool(name="biasp", bufs=2) as bias_pool,
            tc.tile_pool(name="cp", bufs=1) as c_pool,
            tc.tile_pool(name="ysb", bufs=2) as ysb_pool,
            tc.tile_pool(name="psum", bufs=1, space="PSUM") as psum_pool,
        ):
            xr = [xres_pool.tile([P, K_TILES, TOK_C], f8, tag=f"xr{i}", name=f"xr{i}")
                  for i in range(2 if two_x else 1)]

            # PE p-state warmup: dummy DoubleRow matmuls on a memset scratch
            # tile run while the first X/W DMAs are in flight, so the real
            # stream starts at full clock.
            wsc = c_pool.tile([P, 2, OB], f8, tag="wsc", name="wsc")
            nc.vector.memset(wsc[:], 0)
            wps = psum_pool.tile([P, OB], mybir.dt.float32, name="ps0")
            for _ in range(6):
                nc.tensor.matmul(
                    out=wps[:], lhsT=wsc[:, :, 0:P], rhs=wsc[:],
                    start=True, stop=True,
                    perf_mode=mybir.MatmulPerfMode.DoubleRow,
                )

            def load_w(ob, interleave_x=False):
                """Stage this out-block's full W panel; 8 chunk DMAs."""
                wr = wres_pool.tile([P, K_TILES, OB], f8, name="wr")
                for kg in range(K_GROUPS):
                    sl = slice(kg * KG, (kg + 1) * KG)
                    if interleave_x:
                        if kg == 0:
                            # halve the first chunks so pair 0 unblocks sooner;
                            # W halves ride the scalar queue in parallel with X
                            for h in range(2):
                                hs = slice(2 * h, 2 * h + 2)
                                nc.sync.dma_start(out=xr[0][:, hs, :], in_=xq[0][:, hs, :])
                                nc.scalar.dma_start(out=wr[:, hs, :], in_=wq[ob, 0][:, hs, :])
                            if two_x:
                                nc.sync.dma_start(out=xr[1][:, 0:KG, :], in_=xl[0])
                            continue
                        nc.sync.dma_start(out=xr[0][:, sl, :], in_=xq[kg])
                        if two_x:
                            nc.sync.dma_start(out=xr[1][:, sl, :], in_=xl[kg])
                    nc.sync.dma_start(out=wr[:, sl, :], in_=wq[ob, kg])
                return wr

            c_sb = c_pool.tile([P, M_TILES], mybir.dt.float32, name="c_sb")
            nc.scalar.dma_start(out=c_sb[:], in_=cc[:, :])

            def mm(psum, ksl, m, wr, start, stop):
                lsl = (slice(0, 2), slice(0, P)) if _PROBE == "samelhs" else \
                    (ksl, slice(m * P, (m + 1) * P))
                nc.tensor.matmul(
                    out=psum[:],
                    lhsT=xr[0][:, lsl[0], lsl[1]],
                    rhs=wr[:, ksl, :],
                    start=start,
                    stop=stop and not two_x,
                    perf_mode=mybir.MatmulPerfMode.DoubleRow,
                )
                if two_x:
                    nc.tensor.matmul(
                        out=psum[:],
                        lhsT=xr[1][:, ksl, m * P:(m + 1) * P],
                        rhs=wr[:, ksl, :],
                        start=False,
                        stop=stop,
                        perf_mode=mybir.MatmulPerfMode.DoubleRow,
                    )

            def drain(psums, ysb, bstage, m):
                """(psum + c[token]) + bias into the Y panel."""
                if _DRAIN == "act":
                    # ACT does the PSUM read; DVE reads/writes SBUF only
                    nc.scalar.copy(out=ysb[:, m, :], in_=psums[m][:])
                    nc.vector.scalar_tensor_tensor(
                        out=ysb[:, m, :], in0=ysb[:, m, :], scalar=c_sb[:, m:m + 1],
                        in1=bstage[:], op0=mybir.AluOpType.add, op1=mybir.AluOpType.add,
                    )
                    return
                src = bstage[:] if _PROBE == "nopsum" else psums[m][:]
                nc.vector.scalar_tensor_tensor(
                    out=ysb[:, m, :], in0=src, scalar=c_sb[:, m:m + 1],
                    in1=bstage[:], op0=mybir.AluOpType.add, op1=mybir.AluOpType.add,
                )

            wr_next = load_w(0, interleave_x=True)
            for ob in range(O_BLOCKS):
                osl = slice(ob * OB, (ob + 1) * OB)
                wr = wr_next
                if ob + 1 < O_BLOCKS:
                    wr_next = load_w(ob + 1)

                bstage = bias_pool.tile([P, OB], mybir.dt.float32, name="bstage")
                nc.scalar.dma_start(out=bstage[:], in_=bias[None, osl].to_broadcast([P, OB]))

                psums = [psum_pool.tile([P, OB], mybir.dt.float32, name=f"ps{_m}")
                         for _m in range(M_TILES)]
                ysb = ysb_pool.tile([P, M_TILES, OB], mybir.dt.float32, name="ysb")

                if ob == 0:
                    # X residency streams in k-order: pair-outer, m-inner
                    for gp in range(N_PAIRS):
                        ksl = slice(2 * gp, 2 * gp + 2)
                        for m in range(M_TILES):
                            mm(psums[m], ksl, m, wr, gp == 0, gp == N_PAIRS - 1)
                    for m in range(M_TILES):
                        drain(psums, ysb, bstage, m)
                        if m == M_TILES // 2 - 1:
                            nc.scalar.dma_start(out=y_v[:, 0:4, osl], in_=ysb[:, 0:4, :])
                    nc.scalar.dma_start(out=y_v[:, 4:8, osl], in_=ysb[:, 4:8, :])
                else:
                    # two-bank interleave: the intervening matmul hides PSUM
                    # accumulate turnaround, banks still complete/drain early
                    for mp in range(M_TILES // 2):
                        m0, m1 = 2 * mp, 2 * mp + 1
                        for gp in range(N_PAIRS):
                            ksl = slice(2 * gp, 2 * gp + 2)
                            first, last = gp == 0, gp == N_PAIRS - 1
                            if not two_x:
                                mm(psums[m0], ksl, m0, wr, first, last)
                                mm(psums[m1], ksl, m1, wr, first, last)
                            else:
                                for i, m in ((0, m0), (0, m1), (1, m0), (1, m1)):
                                    nc.tensor.matmul(
                                        out=psums[m][:],
                                        lhsT=xr[i][:, ksl, m * P:(m + 1) * P],
                                        rhs=wr[:, ksl, :],
                                        start=first and i == 0,
                                        stop=last and i == 1,
                                        perf_mode=mybir.MatmulPerfMode.DoubleRow,
                                    )
                        drain(psums, ysb, bstage, m0)
                        drain(psums, ysb, bstage, m1)
                        if mp == 1:
                            nc.scalar.dma_start(out=y_v[:, 0:4, osl], in_=ysb[:, 0:4, :])
                        elif mp == 2:
                            nc.scalar.dma_start(out=y_v[:, 4:6, osl], in_=ysb[:, 4:6, :])
                        elif mp == 3:
                            nc.scalar.dma_start(out=y_v[:, 6:8, osl], in_=ysb[:, 6:8, :])

    nc.compile()
    return nc


def build(mode: str):
    assert mode in ("f32r", "bf16", "bf16x2")
    mm_dt = mybir.dt.float32r if mode == "f32r" else mybir.dt.bfloat16
    KGf = 4
    XKG = 2

    nc = bacc.Bacc(None)
    xt = nc.declare_dram_parameter("xt", [IN_F, TOK_C], mybir.dt.float32, isOutput=False)
    wt = nc.declare_dram_parameter("wt", [IN_F, OUT_F], mybir.dt.bfloat16, isOutput=False)
    bias = nc.declare_dram_parameter("bias", [OUT_F], mybir.dt.float32, isOutput=False)
    y = nc.declare_dram_parameter("y", [TOK_C, OUT_F], mybir.dt.float32, isOutput=True)

    xt_v = xt.rearrange("(kt p) t -> p kt t", p=P)      # [128, 32, 1024]
    wt_v = wt.rearrange("(kt p) o -> p kt o", p=P)      # [128, 32, 4096]
    y_v = y.rearrange("(mt p) o -> p mt o", p=P)        # [128, 8, 4096]

    n_x = 2 if mode == "bf16x2" else 1

    with tile.TileContext(nc) as tc:
        with (
            tc.tile_pool(name="xres", bufs=1) as xres_pool,
            tc.tile_pool(name="xstage", bufs=2) as xstage_pool,
            tc.tile_pool(name="wstage", bufs=3) as wstage_pool,
            tc.tile_pool(name="wb", bufs=3) as wb_pool,
            tc.tile_pool(name="biasp", bufs=1) as bias_pool,
            tc.tile_pool(name="osb", bufs=4) as osb_pool,
            tc.tile_pool(name="psum", bufs=1, space="PSUM") as psum_pool,
        ):
            xr = [
                xres_pool.tile([P, K_TILES, TOK_C], mm_dt, tag=f"xr{i}", name=f"xr{i}")
                for i in range(n_x)
            ]

            def load_x_chunk(kk):
                xs = xstage_pool.tile([P, XKG, TOK_C], mybir.dt.float32, name="xs")
                nc.sync.dma_start(out=xs[:], in_=xt_v[:, kk * XKG:(kk + 1) * XKG, :])
                sl = slice(kk * XKG, (kk + 1) * XKG)
                nc.vector.tensor_scalar(
                    out=xr[0][:, sl, :], in0=xs[:], scalar1=0.0, scalar2=None,
                    op0=mybir.AluOpType.add,
                )
                if mode == "bf16x2":
                    nc.vector.tensor_sub(out=xr[1][:, sl, :], in0=xs[:], in1=xr[0][:, sl, :])

            for ob in range(O_BLOCKS):
                osl = slice(ob * OB, (ob + 1) * OB)

                psums = [psum_pool.tile([P, OB], mybir.dt.float32, name=f"ps{_m}") for _m in range(M_TILES)]

                for kg in range(K_TILES // KGf):
                    ckg = KGf // XKG
                    if ob == 0:
                        load_x_chunk(kg * ckg)
                    ws = wstage_pool.tile([P, KGf, OB], mybir.dt.bfloat16, name="ws")
                    nc.sync.dma_start(out=ws[:], in_=wt_v[:, kg * KGf:(kg + 1) * KGf, osl])
                    if ob == 0:
                        for jj in range(1, ckg):
                            load_x_chunk(kg * ckg + jj)
                    wb = wb_pool.tile([P, KGf, OB], mm_dt, name="wb")
                    nc.vector.tensor_scalar(
                        out=wb[:], in0=ws[:], scalar1=0.0, scalar2=None,
                        op0=mybir.AluOpType.is_gt,
                    )
                    for ks in range(KGf):
                        k = kg * KGf + ks
                        for m in range(M_TILES):
                            nc.tensor.matmul(
                                out=psums[m][:],
                                lhsT=xr[0][:, k, m * P:(m + 1) * P],
                                rhs=wb[:, ks, :],
                                start=(k == 0),
                                stop=(k == K_TILES - 1) if mode != "bf16x2" else False,
                            )
                            if mode == "bf16x2":
                                nc.tensor.matmul(
                                    out=psums[m][:],
                                    lhsT=xr[1][:, k, m * P:(m + 1) * P],
                                    rhs=wb[:, ks, :],
                                    start=False,
                                    stop=(k == K_TILES - 1),
                                )

                bstage = bias_pool.tile([P, OB], mybir.dt.float32, tag="bstage", name="bstage")
                nc.sync.dma_start(out=bstage[:], in_=bias[None, osl].to_broadcast([P, OB]))
                bias_bc = bias_pool.tile([P, OB], mybir.dt.float32, tag="bbc", name="bias_bc")
                nc.scalar.copy(out=bias_bc[:], in_=bstage[:])

                for m in range(M_TILES):
                    o_sb = osb_pool.tile([P, OB], mybir.dt.float32, name="o_sb")
                    nc.scalar.copy(out=o_sb[:], in_=psums[m][:])
                    nc.vector.tensor_add(out=o_sb[:], in0=o_sb[:], in1=bias_bc[:])
                    nc.sync.dma_start(out=y_v[:, m, osl], in_=o_sb[:])

    nc.compile()
    return nc


def _swizzle_x(xt8):
    """[IN_F, TOK_C] fp8 -> [K_GROUPS, P, KG, TOK_C] (contiguous DMA chunks)."""
    return np.ascontiguousarray(
        xt8.reshape(K_GROUPS, KG, P, TOK_C).transpose(0, 2, 1, 3))


def kernel(X: np.ndarray, weight: np.ndarray, bias: np.ndarray) -> np.ndarray:
    assert X.shape == (N_TOKENS, IN_F) and weight.shape == (OUT_F, IN_F)
    mode = _MODE

    ckey = (mode, _PROBE, _DRAIN)
    if ckey not in _CACHED:
        _CACHED[ckey] = build_fp8(mode) if mode.startswith("fp8") else build(mode)
    nc = _CACHED[ckey]

    if _TRACE:
        _install_ntff_shim()

    import ml_dtypes
    f8 = ml_dtypes.float8_e4m3
    bias_np = np.ascontiguousarray(bias.astype(np.float32, copy=False))

    if mode.startswith("fp8"):
        wbin = (weight > 0)
        if mode == "fp8":
            wv = np.where(wbin, np.float32(0.5), np.float32(-0.5))
        else:
            wv = wbin.astype(np.float32)
        # W^T [in, out] -> [ob, kg, p, j, o]
        wq_np = np.ascontiguousarray(
            wv.T.reshape(K_GROUPS, KG, P, O_BLOCKS, OB).transpose(3, 0, 2, 1, 4)
        ).astype(f8)
        in_maps = []
        for c in range(N_CORES):
            xs = X[c * TOK_C:(c + 1) * TOK_C, :]
            xt = np.ascontiguousarray(xs.T.astype(np.float32, copy=False))
            hi8 = xt.astype(f8)
            m = {"wq": wq_np, "bias": bias_np, "xq": _swizzle_x(hi8)}
            if mode == "fp8":
                c_np = (0.5 * xs.sum(axis=1, dtype=np.float64)).astype(np.float32)
            else:
                m["xl"] = _swizzle_x((xt - hi8.astype(np.float32)).astype(f8))
                c_np = np.zeros(TOK_C, dtype=np.float32)
            m["cc"] = np.ascontiguousarray(c_np.reshape(M_TILES, P).T)
            in_maps.append(m)
    else:
        wt_np = np.ascontiguousarray(weight.T).astype(ml_dtypes.bfloat16)
        in_maps = []
        for c in range(N_CORES):
            xs = X[c * TOK_C:(c + 1) * TOK_C, :]
            xt_np = np.ascontiguousarray(xs.T.astype(np.float32, copy=False))
            in_maps.append({"xt": xt_np, "wt": wt_np, "bias": bias_np})

    res = run_bass_kernel_spmd(
        nc, in_maps, core_ids=list(range(N_CORES)), trace=_TRACE,
    )
    out = np.concatenate([res.results[c]["y"] for c in range(N_CORES)], axis=0)
    if _TRACE:
        kernel.last_exec_time_ns = res.exec_time_ns
        kernel.last_trace = res.instructions_and_trace
    return out.astype(np.float32, copy=False)
